# revision 47
# baseline (speedup 1.0000x reference)
"""Trainium2 Bass kernel for the ARP torus AR(3) winding loss.

Math: the reference sums, per (n_mc, n_samples) angle sequence, Gaussian
log-probs of AR(3) residuals of wrapped angle diffs over 11 winding
offsets k = -5..5.  The winding sum is analytic:

    sum_k -0.5*((dy + 2pi*k - c)/s)^2  =  -0.5*K/s^2*(dy-c)^2
                                          - 0.5*(2pi)^2*(sum_k k^2)/s^2

(sum_k k = 0, sum_k k^2 = 110), so the whole loss reduces to a weighted
sum of squared AR residuals plus a closed-form constant.  The device
computes sum_t (sqrt(w_d)*dy - sqrt(w_d)*c_d)^2 per (row, dim); the host
adds the constant and does the (tiny) group reduction.

Sharding: data-parallel over n_mc - core i gets mc in [4i, 4i+4); no
cross-core communication (each mc's reduction lives on one core).

Device layout: SBUF tile [128 partitions = (half, seq), free = t*3+d]
loaded with two fully-contiguous DMAs.  Ops: diff (DVE), single-step
range wrap (DVE custom op; valid since |dg| < 3pi, asserted on host),
3x3 fused multiply-add FIR taps (DVE affine_then_add, phi baked as
immediates), fused Square+row-reduce on the scalar engine (ACT).
"""

import os

import numpy as np

N_MC, N_S, T, D = 32, 16, 2048, 3
P = 3
KMAX = 5
K = 2 * KMAX + 1
SUM_K2 = float(KMAX * (KMAX + 1) * (2 * KMAX + 1) // 6 * 2)  # 110
N_CORES = 8
MC_PER_CORE = N_MC // N_CORES  # 4
SEQ = MC_PER_CORE * N_S  # 64 sequences per core
TP = T - 1 - P  # 2044 residuals per sequence
HALF = TP // 2  # 1022 residuals per half-row
GLEN = (HALF + P + 1) * D  # 3078 input elems per row
TWO_PI = 2.0 * np.pi


CHUNKS = int(os.environ.get("ARP_CHUNKS", "4"))
# per-dim tap routing: 'dve' = 3 DVE fused taps; 'mixA' = ACT mult +
# GPS add + 2 DVE taps; 'mixG' = GPS mult + GPS add + 2 DVE taps
TAP_PLAN = os.environ.get("ARP_TAPS", "dve,mixA,mixG").split(",")
# dims whose square+reduce runs on DVE (affine_mul_reduce) vs ACT
SQ_DVE = {
    int(x) for x in os.environ.get("ARP_SQDVE", "").split(",") if x != ""
}
GPS_WRAP = os.environ.get("ARP_GPS_WRAP", "1") == "1"
SUB_GPS_FRAC = float(os.environ.get("ARP_SUB_GPS", "0.25"))
BUFS = int(os.environ.get("ARP_BUFS", "2"))
RING_SPLIT = os.environ.get("ARP_RING", "1") == "1"


def _chunk_bounds():
    """t'-ranges per chunk: [(start, len), ...] covering [0, HALF)."""
    base = (HALF + CHUNKS - 1) // CHUNKS
    out = []
    t = 0
    while t < HALF:
        ln = min(base, HALF - t)
        out.append((t, ln))
        t += ln
    return out


def _build_program(phi, sw, bias):
    """Trace the SPMD Bass program. phi (3,3), sw (3,), bias (3,) baked
    as immediates (parameters are tiny; program is compiled per call).

    Chunked along t' for DMA/compute overlap; work split across DVE
    (diff + fused FIR taps), GPSIMD (wrap via mod, some FIR adds), ACT
    (multiplies + fused square-reduce).
    """
    import concourse.tile as tile
    from concourse import bacc, mybir

    f32 = mybir.dt.float32
    Square = mybir.ActivationFunctionType.Square
    Copy = mybir.ActivationFunctionType.Copy
    nc = bacc.Bacc(
        "TRN2", target_bir_lowering=False, debug=False, num_devices=N_CORES
    )
    g = nc.dram_tensor("g", [128, GLEN], f32, kind="ExternalInput")
    chunks = _chunk_bounds()
    acc_out = nc.dram_tensor(
        "acc", [128, D * len(chunks)], f32, kind="ExternalOutput"
    )

    # per-dim effective bias: with GPS wrap, w' = dx + pi, so
    # dy' = dy + pi*(1 - sum_j phi_dj); fold into the square's bias.
    off = np.pi * (1.0 - phi.sum(1)) if GPS_WRAP else np.zeros(D)
    biasp = bias - sw * off  # Square((dy' )*sw + biasp) == (sw*(dy-c))^2

    with tile.TileContext(nc) as tc:
        with tc.tile_pool(name="main", bufs=BUFS) as pool, tc.tile_pool(
            name="accp", bufs=1
        ) as accpool:
            acc = accpool.tile([128, D * len(chunks)], f32, tag="acc")
            bias_t = accpool.tile([128, D], f32, tag="bias")
            for d in range(D):
                nc.gpsimd.memset(bias_t[:, d : d + 1], float(biasp[d]))
            for ci, (t0, L) in enumerate(chunks):
                GL = (L + P + 1) * D  # loaded elems
                FL = GL - D  # diff count * D
                ring = nc.sync if (ci % 2 == 0 or not RING_SPLIT) else nc.scalar
                x = pool.tile([128, GL], f32, tag="x")
                ring.dma_start(out=x[:], in_=g[:, t0 * D : t0 * D + GL])
                dg = pool.tile([128, FL], f32, tag="dg")
                # diff split between DVE and GPSIMD by column range
                sp = int(FL * (1.0 - SUB_GPS_FRAC)) if SUB_GPS_FRAC > 0 else FL
                nc.vector.tensor_sub(dg[:, :sp], x[:, D : D + sp], x[:, 0:sp])
                if sp < FL:
                    nc.gpsimd.tensor_sub(
                        dg[:, sp:FL], x[:, D + sp : D + FL], x[:, sp:FL]
                    )
                w = pool.tile([128, FL], f32, tag="w")
                if GPS_WRAP:
                    # w' = mod(dg + pi, 2pi) in [0, 2pi)
                    nc.gpsimd.tensor_scalar(
                        w[:], dg[:], float(np.pi), float(TWO_PI),
                        mybir.AluOpType.add, mybir.AluOpType.mod,
                    )
                else:
                    nc.vector.add_range_wrap(
                        w[:], dg[:], 0.0, float(np.pi), float(TWO_PI)
                    )
                wv = w[:].rearrange("p (t d) -> p t d", d=D)  # [128, L+3, D]
                for d in range(D):
                    wk = lambda k: wv[:, k : k + L, d]
                    dy = pool.tile([128, L], f32, tag=f"dy{d}")
                    plan = TAP_PLAN[d]
                    if plan == "dve":
                        ta = pool.tile([128, L], f32, tag=f"ta{d}")
                        tb = pool.tile([128, L], f32, tag=f"tb{d}")
                        nc.vector.affine_then_add(
                            ta[:], wk(2), wk(3), -float(phi[d, 0]), 0.0
                        )
                        nc.vector.affine_then_add(
                            tb[:], wk(1), ta[:], -float(phi[d, 1]), 0.0
                        )
                        nc.vector.affine_then_add(
                            dy[:], wk(0), tb[:], -float(phi[d, 2]), 0.0
                        )
                    else:
                        # mult on ACT or GPS, add on GPS, 2 DVE fused taps
                        m0 = pool.tile([128, L], f32, tag=f"m0{d}")
                        s0 = pool.tile([128, L], f32, tag=f"s0{d}")
                        tb = pool.tile([128, L], f32, tag=f"tb{d}")
                        if plan == "mixA":
                            nc.scalar.activation(
                                m0[:], wk(2), Copy,
                                bias=0.0, scale=-float(phi[d, 0]),
                            )
                        else:
                            nc.gpsimd.tensor_scalar_mul(
                                m0[:], wk(2), -float(phi[d, 0])
                            )
                        nc.gpsimd.tensor_add(s0[:], wk(3), m0[:])
                        nc.vector.affine_then_add(
                            tb[:], wk(1), s0[:], -float(phi[d, 1]), 0.0
                        )
                        nc.vector.affine_then_add(
                            dy[:], wk(0), tb[:], -float(phi[d, 2]), 0.0
                        )
                    aslice = acc[:, ci * D + d : ci * D + d + 1]
                    if d in SQ_DVE:
                        # sum (sw*dy+b)^2 = sum (w_d*dy + 2*sw*b)*dy  [+ N*b^2
                        # folded on host]
                        scr = pool.tile([128, L], f32, tag=f"scr{d}")
                        nc.vector.affine_mul_reduce(
                            scr[:], aslice, dy[:], dy[:],
                            float(sw[d] * sw[d]), float(2.0 * sw[d] * biasp[d]),
                        )
                    else:
                        scr = pool.tile([128, L], f32, tag=f"scr{d}")
                        nc.scalar.activation(
                            scr[:], dy[:], Square,
                            bias=bias_t[:, d : d + 1], scale=float(sw[d]),
                            accum_out=aslice,
                        )
            nc.sync.dma_start(out=acc_out[:, :], in_=acc[:])
    nc.finalize()
    return nc


# ---------------- v3: T-sharded, PE-FIR on host-transposed layout ---------
# Core ci owns dy t-range [256*ci, 256*ci+L_ci), L = 256 (252 for core 7),
# for ALL 512 (mc, s) sequences.  Host transposes each core's g-window into
# layout B: SBUF tiles [128 partitions = flat (t,d) window, 512 rows].
# Tiles overlap by 12 flat positions (stride 116) so the AR(3) band never
# crosses a tile: diff + wrap stay elementwise (partition-shifted), the FIR
# becomes one banded matmul per tile (TensorE, float32r at full rate), the
# square runs on ACT with per-partition scale/bias, and the t-reduction is
# a ones-masked matmul accumulating into PSUM [1, 512].

V3 = os.environ.get("ARP_V2", "0") != "1"
TILE_W = 128  # g-window flat positions per tile
MMK = TILE_W - D  # 125 valid diffs per tile
STRIDE = MMK - (P * D)  # 116 dy outputs per tile
NT = 7  # tiles: STRIDE*6 + TILE_W = 824 >= 780 needed
NROW = N_MC * N_S  # 512 sequences
LMAX = (TP + N_CORES - 1) // N_CORES  # 256
# fp16 input tensor columns: coef16(6) + D-band(125) + psi(3x116) + mask(7)
# + NT tile blocks of NROW
DB0 = 6
PSI0 = DB0 + MMK
MASK0 = PSI0 + 3 * STRIDE
AUXC = MASK0 + NT
RED_DVE = os.environ.get("ARP_RED_DVE", "0") == "1"
SQ_SCALE = 16.0  # sq output scaled by 1/SQ_SCALE^2 to fit fp16; host undoes


def _core_L(ci):
    t0 = ci * LMAX
    return min(LMAX, TP - t0)


def _build_program_v3():
    import concourse.tile as tile
    from concourse import bacc, mybir

    f32 = mybir.dt.float32
    f16 = mybir.dt.float16
    Square = mybir.ActivationFunctionType.Square
    nc = bacc.Bacc(
        "TRN2", target_bir_lowering=False, debug=False, num_devices=N_CORES
    )
    gx = nc.dram_tensor("gx", [128, AUXC + NT * NROW], f16, kind="ExternalInput")
    acc_out = nc.dram_tensor("acc", [1, NROW], f32, kind="ExternalOutput")

    # DMA chunks of k-tiles (chunk 0 carries aux+coef), each on a
    # configurable queue: s=sync HWDGE, a=scalar HWDGE, g=gpsimd SWDGE
    groups = [
        [int(x) for x in grp.split("+")]
        for grp in os.environ.get("ARP_V3_GROUPS", "0,1+2+3,4+5+6").split(",")
    ]
    rings_s = os.environ.get("ARP_V3_RINGS", "s,g,g").split(",")

    with tile.TileContext(nc) as tc:
        with tc.tile_pool(name="xp", bufs=1) as xpool, tc.tile_pool(
            name="work", bufs=3
        ) as pool, tc.tile_pool(name="ps", bufs=2, space="PSUM") as pspool, tc.tile_pool(
            name="red", bufs=1, space="PSUM"
        ) as redpool:
            ring_map = {"s": nc.sync, "a": nc.scalar, "g": nc.gpsimd}
            xts = []
            for gi, ks in enumerate(groups):
                k0, k1 = ks[0], ks[-1] + 1
                c0 = AUXC + k0 * NROW if k0 > 0 else 0
                xt = xpool.tile([128, AUXC + (k1 - k0) * NROW if k0 == 0 else k1 * NROW - k0 * NROW], f16, tag=f"x{gi}")
                ring_map[rings_s[gi % len(rings_s)]].dma_start(
                    out=xt[:], in_=gx[:, c0 : AUXC + k1 * NROW]
                )
                xts.append((xt, c0, k0, k1))

            aux = xts[0][0]  # chunk 0 starts with the aux columns

            def kview(k):
                for xt, c0, k0, k1 in xts:
                    if k0 <= k < k1:
                        off = AUXC + k * NROW - c0
                        return xt[:, off : off + NROW]
                raise AssertionError

            coef = xpool.tile([128, 6], f32, tag="coef")
            nc.vector.tensor_scalar_add(coef[:], aux[:, 0:DB0], 0.0)

            red = redpool.tile([1, NROW], f32, tag="red")
            for k in range(NT):
                q = (STRIDE * k) % D
                xk = kview(k)
                # diff on PE: dg[m] = g[m+3] - g[m] via +-1 band
                dgp = pspool.tile([128, NROW], f32, tag="dgp")
                nc.tensor.matmul(
                    dgp[0:MMK, :], aux[0:TILE_W, DB0 : DB0 + MMK], xk[0:TILE_W, :],
                    start=True, stop=True,
                )
                # dx = single-step wrap of dg into [-pi, pi], fp16 for PE FIR
                w = pool.tile([128, NROW], f16, tag="w")
                nc.vector.add_range_wrap(
                    w[0:MMK, :], dgp[0:MMK, :], 0.0, float(np.pi), float(TWO_PI)
                )
                dyp = pspool.tile([128, NROW], f32, tag="dyp")
                nc.tensor.matmul(
                    dyp[0:STRIDE, :],
                    aux[0:MMK, PSI0 + q * STRIDE : PSI0 + (q + 1) * STRIDE],
                    w[0:MMK, :],
                    start=True, stop=True,
                )
                # sq = ((sw_d/16)*dy' + biasp_d/16)^2, fp16 for the PE reduce
                sq = pool.tile([128, NROW], f16, tag="sq")
                nc.scalar.activation(
                    sq[0:STRIDE, :], dyp[0:STRIDE, :], Square,
                    bias=coef[0:STRIDE, 2 * q + 1 : 2 * q + 2],
                    scale=coef[0:STRIDE, 2 * q : 2 * q + 1],
                )
                nc.tensor.matmul(
                    red[0:1, :],
                    aux[0:STRIDE, MASK0 + k : MASK0 + k + 1],
                    sq[0:STRIDE, :],
                    start=(k == 0), stop=(k == NT - 1),
                )
            out_sb = pool.tile([1, NROW], f32, tag="osb")
            if RED_DVE:
                nc.vector.tensor_scalar_add(out_sb[0:1, :], red[0:1, :], 0.0)
            else:
                nc.scalar.copy(out_sb[0:1, :], red[0:1, :])
            nc.sync.dma_start(out=acc_out[:, :], in_=out_sb[0:1, :])
    nc.finalize()
    return nc


def _v3_inputs(g, phi, sw, biasp):
    """Per-core [128, AUXC + NT*NROW] fp16 input: coef, D, psi, mask, tiles."""
    gf = np.ascontiguousarray(g.reshape(NROW, T * D))
    aux = np.zeros((128, AUXC), np.float16)
    for q in range(3):
        dd = (np.arange(128) + q) % D
        aux[:, 2 * q] = (sw[dd] / SQ_SCALE).astype(np.float16)
        aux[:, 2 * q + 1] = (biasp[dd] / SQ_SCALE).astype(np.float16)
    for m in range(MMK):
        aux[m, DB0 + m] = -1.0
        aux[m + D, DB0 + m] = 1.0
    for q in range(3):
        for m in range(STRIDE):
            d = (q + m) % D
            col = PSI0 + q * STRIDE + m
            aux[m + 9, col] = 1.0
            aux[m + 6, col] = -phi[d, 0]
            aux[m + 3, col] = -phi[d, 1]
            aux[m, col] = -phi[d, 2]
    ins = []
    for ci in range(N_CORES):
        L = _core_L(ci)
        t0 = ci * LMAX
        span = 3 * (min(t0 + L + P + 1, T) - t0)
        window = np.zeros((NROW, STRIDE * (NT - 1) + TILE_W), np.float16)
        window[:, :span] = gf[:, 3 * t0 : 3 * t0 + span]
        buf = np.zeros((128, AUXC + NT * NROW), np.float16)
        buf[:, :AUXC] = aux
        for k in range(NT):
            vk = max(0, min(STRIDE, 3 * L - STRIDE * k))
            buf[:vk, MASK0 + k] = 1.0
            buf[:, AUXC + k * NROW : AUXC + (k + 1) * NROW] = window[
                :, STRIDE * k : STRIDE * k + TILE_W
            ].T
        ins.append({"gx": buf})
    return ins


def kernel(g, ar_phi, ar_eta, ar_c):
    g = np.ascontiguousarray(np.asarray(g, dtype=np.float32))
    assert g.shape == (N_MC, N_S, T, D), g.shape
    if V3:
        return _kernel_v3(g, ar_phi, ar_eta, ar_c)
    return _kernel_v2(g, ar_phi, ar_eta, ar_c)


def predict_exec_ns(g, ar_phi, ar_eta, ar_c):
    """Per-core exec-time estimate from the Tile cost model (CoreSim
    virtual clock) — used when NTFF profiling is unavailable."""
    g = np.ascontiguousarray(np.asarray(g, dtype=np.float32))
    phi = np.asarray(ar_phi, np.float64)
    s = np.abs(np.asarray(ar_eta, np.float64))
    c = np.asarray(ar_c, np.float64)
    sw = np.sqrt(0.5 * K / s**2)
    biasp = -sw * c
    nc = _build_program_v3()
    in_maps = _v3_inputs(g, phi, sw, biasp)
    from concourse.bass_interp import CoreSim

    sim = CoreSim(nc)
    for nm, v in in_maps[0].items():
        sim.tensor(nm)[:] = v
    sim.simulate()
    return int(sim.time)


def _kernel_v3(g, ar_phi, ar_eta, ar_c):
    phi = np.asarray(ar_phi, np.float64)
    s = np.abs(np.asarray(ar_eta, np.float64))
    c = np.asarray(ar_c, np.float64)
    w_d = 0.5 * K / s**2
    sw = np.sqrt(w_d)
    biasp = -sw * c  # single-step wrap yields true dx

    # single-step wrap validity (holds with big margin for N(0,1) angles)
    dgmax = float(np.abs(np.diff(g.reshape(-1, T, D), axis=1)).max())
    assert dgmax < 3 * np.pi, f"|dg| max {dgmax} >= 3pi; 1-step wrap invalid"

    nc = _build_program_v3()
    in_maps = _v3_inputs(g, phi, sw, biasp)

    if os.environ.get("ARP_SIM"):
        from concourse.bass_interp import CoreSim

        accs = []
        for ci in range(int(os.environ.get("ARP_SIM_CORES", "1"))):
            sim = CoreSim(nc)
            for nm, v in in_maps[ci].items():
                sim.tensor(nm)[:] = v
            sim.simulate()
            accs.append(np.array(sim.tensor("acc"), np.float64))
        while len(accs) < N_CORES:
            accs.append(accs[-1])
        kernel.last_exec_ns = None
    else:
        from concourse.bass_utils import run_bass_kernel_spmd

        res = run_bass_kernel_spmd(nc, in_maps, list(range(N_CORES)))
        kernel.last_results = res
        accs = [np.asarray(res.results[ci]["acc"], np.float64) for ci in range(N_CORES)]
        kernel.last_exec_ns = res.exec_time_ns

    const_d = (
        -0.5 * TWO_PI**2 * SUM_K2 / s**2 - K * np.log(s) - 0.5 * K * np.log(TWO_PI)
    )
    const_total = N_S * TP * const_d.sum()
    per_seq = np.zeros(NROW, np.float64)
    for ci in range(N_CORES):
        per_seq += accs[ci][0]
    per_seq *= SQ_SCALE * SQ_SCALE  # undo the fp16 range scaling
    per_mc = per_seq.reshape(N_MC, N_S).sum(1)
    return (const_total - per_mc).astype(np.float32)


def _kernel_v2(g, ar_phi, ar_eta, ar_c):
    phi = np.asarray(ar_phi, np.float64)
    s = np.abs(np.asarray(ar_eta, np.float64))
    c = np.asarray(ar_c, np.float64)

    w_d = 0.5 * K / s**2
    sw = np.sqrt(w_d)
    bias = -sw * c

    if not GPS_WRAP:
        # Single-step wrap validity (holds with big margin for N(0,1) angles).
        dgmax = float(np.abs(np.diff(g.reshape(-1, T, D), axis=1)).max())
        assert dgmax < 3 * np.pi, f"|dg| max {dgmax} >= 3pi; 1-step wrap invalid"

    nc = _build_program(phi, sw, bias)
    gr = g.reshape(N_MC, N_S * T * D)
    in_maps = []
    for ci in range(N_CORES):
        gs = gr[ci * MC_PER_CORE : (ci + 1) * MC_PER_CORE].reshape(SEQ, T * D)
        gx = np.empty((128, GLEN), np.float32)
        for h in range(2):
            gx[h * SEQ : (h + 1) * SEQ] = gs[:, h * HALF * D : h * HALF * D + GLEN]
        in_maps.append({"g": gx})

    if os.environ.get("ARP_SIM"):
        from concourse.bass_interp import CoreSim

        accs = []
        for ci in range(int(os.environ.get("ARP_SIM_CORES", "1"))):
            sim = CoreSim(nc)
            sim.tensor("g")[:] = in_maps[ci]["g"]
            sim.simulate()
            accs.append(np.array(sim.tensor("acc"), np.float64))
        # replicate core 0 result for remaining cores (sim-only smoke path)
        while len(accs) < N_CORES:
            accs.append(accs[-1])
        exec_ns = None
    else:
        from concourse.bass_utils import run_bass_kernel_spmd

        res = run_bass_kernel_spmd(
            nc,
            in_maps,
            list(range(N_CORES)),
            trace=bool(os.environ.get("ARP_TRACE")),
        )
        kernel.last_results = res
        accs = [np.asarray(res.results[ci]["acc"], np.float64) for ci in range(N_CORES)]
        exec_ns = res.exec_time_ns
    kernel.last_exec_ns = exec_ns

    const_d = -0.5 * TWO_PI**2 * SUM_K2 / s**2 - K * np.log(s) - 0.5 * K * np.log(TWO_PI)
    const_total = N_S * TP * const_d.sum()
    # DVE affine_mul_reduce squares omit the constant b^2 term per element
    off = np.pi * (1.0 - phi.sum(1)) if GPS_WRAP else np.zeros(D)
    biasp = bias - sw * off
    for d in SQ_DVE:
        const_total -= N_S * TP * float(biasp[d]) ** 2
    out = np.empty(N_MC, np.float64)
    for ci in range(N_CORES):
        rows = accs[ci].sum(1)  # [128] (sums dims and chunks)
        per_seq = rows[:SEQ] + rows[SEQ:]  # halves
        per_mc = per_seq.reshape(MC_PER_CORE, N_S).sum(1)
        out[ci * MC_PER_CORE : (ci + 1) * MC_PER_CORE] = const_total - per_mc
    return out.astype(np.float32)
